# revision 14
# baseline (speedup 1.0000x reference)
"""Trainium2 Bass kernel for nn_DCGN_78967268704510.

Math: the reference's get_adjacent() builds a diagonal matrix (the faithful
buggy triple loop zeroes every off-diagonal), adds I, then symmetric-
normalizes; for a diagonal matrix D^-1/2 A D^-1/2 == I exactly (to fp32
ulps).  attn_pool feeds only get_adjacent, so the whole network collapses
to two fused stages:

  h   = leaky( (sum_p x[:,4s+p,:] * conv1_w[p,:]) @ prop1_W + prop1_B )
  out = leaky( (sum_p h[:,4t+p,:] * conv2_w[p,:]) @ prop2_W + prop2_B )

Verified vs the full reference: absmax err 8e-08 (2.7e-6 of output scale).

Sharding: pure data parallel, batch 64 -> 8 cores x 8 batches.

Per-core pipeline (per batch b):
  - DMA x[b] as 4 row tiles c_q [128n, 2048f]
  - DVE: c_q *= replicated conv1_w pattern (writes float32r)
  - PE pool: G.T @ c_q with G[n,t]=1 iff t==n//4 stationary (M=32 col-
    tiled at partition 32q, N=512 chunks, fp32r 1cyc/row) -> xc [128s, 2048f]
  - PE transpose (identity) 16x [128,128] -> xcT [f, s] (float32r)
  - mm1: 16 K-tile fp32r matmuls vs prop1_W -> h psum
  - +bias (DVE), LeakyRelu (ACT), *conv2_w pattern (DVE, float32r out)
  - stage-2 pool-transpose matmuls (fp32r), batched-by-4 mm2 vs prop2_W,
    +bias, leaky, DMA out.
"""
import sys

if '/opt/trn_rl_repo' not in sys.path:
    sys.path.insert(0, '/opt/trn_rl_repo')

import numpy as np

import concourse.bass as bass
import concourse.mybir as mybir
import concourse.tile as tile
from concourse.bass_utils import run_bass_kernel_spmd
from concourse.vector_clock import ScopedClock

N_CORES = 8
B, N, F, HID, NCLASS, P = 64, 512, 2048, 1100, 512, 4
BPC = B // N_CORES          # 8 batches per core
S = N // P                  # 128 stage-1 nodes
T = S // P                  # 32 stage-2 nodes
FT = F // 128               # 16 f-tiles
JT = (HID + 127) // 128     # 9 j-tiles, last has 76 rows
JLAST = HID - 128 * (JT - 1)
MM1_CHUNKS = (384, 384, 332)   # all >=256 so float32r runs at 1 cyc/row

FP32 = mybir.dt.float32
F32R = mybir.dt.float32r


class PatchedTileContext(tile.TileContext):
    """This container's walrus refuses ANY instruction carrying >1 sync
    wait (the TPB EVENTS struct has a single wait slot and the codegen
    won't split).  Split every multi-wait instruction into single-wait
    same-engine nops followed by the instruction with its last wait."""

    def _split_waits(self, inst):
        si = inst.sync_info
        waits = list(si.on_wait) if si and si.on_wait else []
        if len(waits) <= 1:
            return
        for w in waits[:-1]:
            nop = mybir.InstNoOp(
                name=self.nc.get_next_instruction_name(), ins=[], outs=[]
            )
            nop.engine = inst.engine
            nop.sync_info = mybir.SyncInfo(on_wait=[w], on_update=[])
            nop.bass_nofuse = True
            self._add_instruction(nop)
        inst.sync_info = mybir.SyncInfo(
            on_wait=[waits[-1]], on_update=list(si.on_update or [])
        )

    def _commit_instruction(self, inst, lazy_reg_writes=True):
        if inst.engine != mybir.EngineType.Unassigned:
            self._split_waits(inst)
        return super()._commit_instruction(inst, lazy_reg_writes)

    def _drain_and_barrier(self, tick_clock, wait_clock):
        probe = self.nc.sync.nop()
        wait_clock.add_sem_waits(
            probe.ins, ScopedClock({None: tick_clock.global_clock})
        )
        si = probe.ins.sync_info
        waits = list(si.on_wait) if si and si.on_wait else []
        if si and waits:
            probe.ins.sync_info = mybir.SyncInfo(
                on_wait=waits[:1], on_update=list(si.on_update or [])
            )
        for w in waits[1:]:
            n2 = self.nc.sync.nop()
            n2.ins.sync_info = mybir.SyncInfo(on_wait=[w], on_update=[])
        self.nc.sync.drain()
        self.nc.all_engine_barrier()
        assert self.sems is not None
        popped = self.nc._tile_sem_poison_stack.pop()
        assert popped is self._sem_poison
        self.nc.clear_and_free_semaphores(list(self.sems.allocated().values()))
        self.nc.all_engine_barrier()


def build_nc():
    nc = bass.Bass()
    xs_d = nc.dram_tensor('xs', [BPC, N, F], F32R, kind='ExternalInput')
    w1rep_d = nc.dram_tensor('w1rep', [128, F], FP32, kind='ExternalInput')
    g0_d = nc.dram_tensor('g0', [128, T], F32R, kind='ExternalInput')
    g4_d = nc.dram_tensor('g4', [128, 512], F32R, kind='ExternalInput')
    id_d = nc.dram_tensor('ident', [128, 128], F32R, kind='ExternalInput')
    w1p_d = nc.dram_tensor('w1p', [F, HID], F32R, kind='ExternalInput')
    b1rep_d = nc.dram_tensor('b1rep', [128, HID], FP32, kind='ExternalInput')
    w2rep_d = nc.dram_tensor('w2rep', [128, HID], FP32, kind='ExternalInput')
    w2p_d = nc.dram_tensor('w2p', [HID, NCLASS], F32R, kind='ExternalInput')
    b2rep_d = nc.dram_tensor('b2rep', [128, NCLASS], FP32, kind='ExternalInput')
    y_d = nc.dram_tensor('y', [BPC, T, NCLASS], FP32, kind='ExternalOutput')
    y_flat = y_d.rearrange('b t c -> (b t) c')   # [256, 512]

    with PatchedTileContext(nc) as tc:
        with (
            tc.tile_pool(name='wpool', bufs=1) as wpool,
            tc.tile_pool(name='cpool', bufs=6) as cpool,
            tc.tile_pool(name='xcpool', bufs=2) as xcpool,
            tc.tile_pool(name='xcTpool', bufs=2) as xcTpool,
            tc.tile_pool(name='hpool', bufs=2) as hpool,
            tc.tile_pool(name='h2pool', bufs=2) as h2pool,
            tc.tile_pool(name='hcTpool', bufs=2) as hcTpool,
            tc.tile_pool(name='opool', bufs=2) as opool,
            tc.tile_pool(name='pbig', bufs=1, space='PSUM') as pbigpool,
            tc.tile_pool(name='ph', bufs=3, space='PSUM') as phpool,
            tc.tile_pool(name='p2', bufs=1, space='PSUM') as p2pool,
        ):
            # ---- constants / weights (loaded once) ----
            w1rep = wpool.tile([128, F], FP32, tag='w1rep')
            nc.sync.dma_start(out=w1rep[:], in_=w1rep_d[:])
            g0 = wpool.tile([128, T], F32R, tag='g0')
            nc.sync.dma_start(out=g0[:], in_=g0_d[:])
            g4 = wpool.tile([128, 512], F32R, tag='g4')
            nc.sync.dma_start(out=g4[:], in_=g4_d[:])
            ident = wpool.tile([128, 128], F32R, tag='ident')
            nc.sync.dma_start(out=ident[:], in_=id_d[:])
            b1rep = wpool.tile([128, HID], FP32, tag='b1rep')
            nc.sync.dma_start(out=b1rep[:], in_=b1rep_d[:])
            w2rep = wpool.tile([128, HID], FP32, tag='w2rep')
            nc.sync.dma_start(out=w2rep[:], in_=w2rep_d[:])
            b2rep = wpool.tile([128, NCLASS], FP32, tag='b2rep')
            nc.sync.dma_start(out=b2rep[:], in_=b2rep_d[:])
            w1p = wpool.tile([128, FT * HID], F32R, tag='w1p')
            for k in range(FT):
                nc.sync.dma_start(
                    out=w1p[:, k * HID:(k + 1) * HID],
                    in_=w1p_d[k * 128:(k + 1) * 128, :],
                )
            w2p = wpool.tile([128, JT * NCLASS], F32R, tag='w2p')
            for m in range(JT):
                rows = 128 if m < JT - 1 else JLAST
                nc.sync.dma_start(
                    out=w2p[0:rows, m * NCLASS:(m + 1) * NCLASS],
                    in_=w2p_d[m * 128:m * 128 + rows, :],
                )

            hcT = [None, None]
            for b in range(BPC):
                # ---- load + scale x (scale writes rounded f32r) ----
                cq = []
                for q in range(4):
                    c = cpool.tile([128, F], F32R, tag='c')
                    nc.sync.dma_start(
                        out=c[:], in_=xs_d[b, q * 128:(q + 1) * 128, :]
                    )
                    cq.append(c)
                for q in range(4):
                    eng = nc.vector if q < 3 else nc.gpsimd
                    eng.tensor_mul(cq[q][:], cq[q][:], w1rep[:])

                # ---- pool: xc[s, f] = G.T @ c_q at col-group 32q ----
                pbig = pbigpool.tile([128, F], FP32, tag='pbig')
                for ch in range(4):
                    for q in range(4):
                        nc.tensor.matmul(
                            pbig[:, 512 * ch:512 * (ch + 1)],
                            g4[:, 128 * q:128 * (q + 1)],
                            cq[q][:, 512 * ch:512 * (ch + 1)],
                            start=(q == 0), stop=(q == 3),
                        )
                xc = xcpool.tile([128, F], F32R, tag='xc')
                for ch in range(2):
                    nc.scalar.copy(
                        out=xc[:, 1024 * ch:1024 * (ch + 1)],
                        in_=pbig[:, 1024 * ch:1024 * (ch + 1)],
                    )

                # ---- transpose via normal-mode matmul against identity
                #      (keeps PE HAM-warm; reuses pbig banks) ----
                xcT = xcTpool.tile([128, F], F32R, tag='xcT')
                for ch in range(4):
                    for kk in range(4):
                        k = 4 * ch + kk
                        nc.tensor.matmul(
                            pbig[:, 512 * ch + 128 * kk:512 * ch + 128 * (kk + 1)],
                            xc[:, k * 128:(k + 1) * 128],
                            ident[:],
                            start=True, stop=True,
                        )
                for ch in range(2):
                    nc.scalar.copy(
                        out=xcT[:, 1024 * ch:1024 * (ch + 1)],
                        in_=pbig[:, 1024 * ch:1024 * (ch + 1)],
                    )

                # ---- mm1: h = xcT.T @ prop1_W  (fp32r) ----
                ph = []
                c0 = 0
                for cn in MM1_CHUNKS:
                    pht = phpool.tile([128, cn], FP32, tag='ph')
                    for k in range(FT):
                        nc.tensor.matmul(
                            pht[:],
                            xcT[:, k * 128:(k + 1) * 128],
                            w1p[:, k * HID + c0:k * HID + c0 + cn],
                            start=(k == 0),
                            stop=(k == FT - 1),
                        )
                    ph.append((pht, c0, cn))
                    c0 += cn

                # ---- epilogue 1: h2 = leaky(h + b1) * w2pattern ----
                h = hpool.tile([128, HID], FP32, tag='h')
                for pht, c0, cn in ph:
                    nc.vector.tensor_add(
                        h[:, c0:c0 + cn], pht[:], b1rep[:, c0:c0 + cn]
                    )
                nc.scalar.activation(
                    h[:], h[:], mybir.ActivationFunctionType.Lrelu, alpha=0.01
                )
                h2 = h2pool.tile([128, HID], F32R, tag='h2')
                nc.vector.tensor_mul(h2[:], h[:], w2rep[:])

                # ---- stage-2 pool-transpose: hcT[j, (b%4)*32 + t] ----
                pt2 = p2pool.tile([128, JT * T], FP32, tag='p2', name=f'pt2_{b}')
                for m in range(JT):
                    rows = 128 if m < JT - 1 else JLAST
                    nc.tensor.matmul(
                        pt2[0:rows, m * T:(m + 1) * T],
                        h2[:, m * 128:m * 128 + rows],
                        g0[:],
                        start=True, stop=True,
                    )
                g, bg = divmod(b, 4)
                if bg == 0:
                    hcT[g] = hcTpool.tile(
                        [128, JT * 128], F32R, tag='hcT', name=f'hcT{g}'
                    )
                dst = hcT[g].rearrange('p (m c) -> p m c', m=JT)[
                    :, :, 32 * bg:32 * (bg + 1)
                ]
                nc.scalar.copy(
                    out=dst, in_=pt2[:].rearrange('p (m c) -> p m c', m=JT)
                )

                if bg == 3:
                    # ---- mm2 for this 4-batch group (fp32r) ----
                    po = p2pool.tile([128, NCLASS], FP32, tag='p2', name=f'po_{g}')
                    for m in range(JT):
                        rows = 128 if m < JT - 1 else JLAST
                        nc.tensor.matmul(
                            po[:],
                            hcT[g][0:rows, m * 128:(m + 1) * 128],
                            w2p[0:rows, m * NCLASS:(m + 1) * NCLASS],
                            start=(m == 0),
                            stop=(m == JT - 1),
                        )
                    ob = opool.tile([128, NCLASS], FP32, tag='ob')
                    nc.vector.tensor_add(ob[:], po[:], b2rep[:])
                    nc.scalar.activation(
                        ob[:], ob[:],
                        mybir.ActivationFunctionType.Lrelu, alpha=0.01,
                    )
                    nc.sync.dma_start(
                        out=y_flat[128 * g:128 * (g + 1), :], in_=ob[:]
                    )
    return nc


def _host_consts(conv1_w, pool1_w, pool1_b, prop1_W, prop1_B,
                 conv2_w, pool2_w, pool2_b, prop2_W, prop2_B):
    f32 = lambda a: np.ascontiguousarray(np.asarray(a, dtype=np.float32))
    g0 = np.zeros((128, T), dtype=np.float32)
    g0[np.arange(128), np.arange(128) // 4] = 1.0
    g4 = np.zeros((128, 512), dtype=np.float32)
    for q in range(4):
        g4[np.arange(128), 128 * q + 32 * q + np.arange(128) // 4] = 1.0
    return {
        'w1rep': f32(np.tile(np.asarray(conv1_w), (32, 1))),
        'g0': g0,
        'g4': g4,
        'ident': np.eye(128, dtype=np.float32),
        'w1p': f32(prop1_W),
        'b1rep': f32(np.broadcast_to(np.asarray(prop1_B), (128, HID))),
        'w2rep': f32(np.tile(np.asarray(conv2_w), (32, 1))),
        'w2p': f32(prop2_W),
        'b2rep': f32(np.broadcast_to(np.asarray(prop2_B), (128, NCLASS))),
    }


_COMPILED = {}


def run_on_cores(inputs, trace=False, **run_kwargs):
    x = np.ascontiguousarray(np.asarray(inputs['x'], dtype=np.float32))
    consts = _host_consts(**{k: v for k, v in inputs.items()
                             if k not in ('x', 'pooling_size')})
    if 'nc' not in _COMPILED:
        _COMPILED['nc'] = build_nc()
    nc = _COMPILED['nc']
    in_maps = []
    for c in range(N_CORES):
        m = {'xs': np.ascontiguousarray(x[c * BPC:(c + 1) * BPC])}
        m.update(consts)
        in_maps.append(m)
    res = run_bass_kernel_spmd(
        nc, in_maps, core_ids=list(range(N_CORES)), trace=trace, **run_kwargs
    )
    out = np.concatenate([res.results[c]['y'] for c in range(N_CORES)], axis=0)
    return out, res


def kernel(**inputs):
    out, _ = run_on_cores(inputs)
    return out
